# revision 3
# baseline (speedup 1.0000x reference)
"""Trainium2 Bass kernel for ErnieImageAttention (non-causal MHA with per-head
RMSNorm on q/k + rotary embedding), tensor-parallel over heads on 8 NeuronCores.

Sharding: 16 heads / 8 cores = 2 heads per core. Each core computes its heads'
q/k/v projections, attention, and a partial output projection (row-parallel
Wo); the host sums the 8 partials and adds the bias.

Per-core dataflow (S=4096, D=2048, Hd=128, 2 local heads):
  phase 1 (per 128-row s-tile):
    q/k/v = hiddenT-chunk matmuls (bf16, N=256 = both heads) accumulated in
    PSUM, emitted proj-major (all q, then k, then v) so tile-0 compute starts
    as soon as Wq lands; RMSNorm stats via ONE fused DVE scalar_tensor_tensor
    (x*x with accum_out) per head; RoPE via host-precomputed bf16 coefficient
    tables (g gains and the 1/sqrt(Hd) logit scale folded in) with the 1/rms
    factor fused into the same DVE ops (scalar operand); q/k transposed to
    [d, s] via DMA-XBAR transposes issued on the ACT hwdge queue (no PE
    transposes, no ACT copies); v kept [s, d] bf16.
  phase 2 (per q-block of 512 cols, per head, k-tiles in groups of 3/2):
    scoresT[k,q] = kT.T @ qT (bf16) into two ping-pong PSUM regions (3+2
    banks); one wide Exp per group on ACT (the phase-2 bottleneck engine);
    group sums accumulated into a 3-lane bf16 acc on DVE; denominator via
    GpSimd partition_all_reduce (no PE work), reciprocal + normalization on
    DVE; attn_T[d,q] = sum_k V[k,d]^T expT[k,q] accumulated in PSUM (po,
    2 banks ping-pong across heads).
  phase 3 (drip-fed between k-groups): fin[s, :2048] = sum_h attnT_h.T @
    WoT_h (f32r) in a single dedicated PSUM bank, drained on DVE, DMA'd out.

Softmax is max-subtraction-free: logits are ~N(0,1) by construction
(RMSNorm'd q/k, 1/sqrt(Hd) folded into q's rope tables).
"""

import numpy as np
import ml_dtypes

import concourse.bass as bass
import concourse.tile as tile
from concourse import bacc, mybir, bass_isa
from concourse import bass_utils

F32 = mybir.dt.float32
F32R = mybir.dt.float32r
BF16 = mybir.dt.bfloat16
AX = mybir.AxisListType
AF = mybir.ActivationFunctionType
ALU = mybir.AluOpType

S = 4096
D = 2048
HD = 128
HEADS = 16
NCORES = 8
HLOC = HEADS // NCORES  # 2 heads per core
DLOC = HLOC * HD  # 256 local head dims
CH = D // 128  # 16 contraction chunks for projections
EPS = 1e-5
SCL = 1.0 / np.sqrt(HD)

QCOLS = 512  # q columns per attention block


def build(nc, tc, io, s_len):
    st_n = s_len // 128  # s tiles
    qb_n = s_len // QCOLS  # q blocks
    qb_st = QCOLS // 128  # s tiles per q block
    kt_n = st_n  # k tiles

    ht, wq, wk, wv, wo, cgq, sgq, cgk, sgk, out = (
        io["ht"], io["wq"], io["wk"], io["wv"], io["wo"],
        io["cgq"], io["sgq"], io["cgk"], io["sgk"], io["out"],
    )

    import contextlib

    with contextlib.ExitStack() as ctx:
        ctx.enter_context(nc.allow_low_precision(
            reason="bf16/f32r compute; values are O(1) and the rel-err "
                   "budget is 2e-2"))
        consts = ctx.enter_context(tc.tile_pool(name="consts", bufs=1))
        persist = ctx.enter_context(tc.tile_pool(name="persist", bufs=1))
        ht_pool = ctx.enter_context(tc.tile_pool(name="ht", bufs=2))
        cs_pool = ctx.enter_context(tc.tile_pool(name="cs", bufs=2))
        work = ctx.enter_context(tc.tile_pool(name="work", bufs=2))
        et_pool = ctx.enter_context(tc.tile_pool(name="et", bufs=4))
        at_pool = ctx.enter_context(tc.tile_pool(name="at", bufs=6))
        acc_pool = ctx.enter_context(tc.tile_pool(name="acc", bufs=2))
        den_pool = ctx.enter_context(tc.tile_pool(name="den", bufs=2))
        fin_pool = ctx.enter_context(tc.tile_pool(name="fin", bufs=6))

        eps_t = consts.tile([128, 1], F32)
        nc.vector.memset(eps_t[:], EPS)

        # Startup ordering: the first q-projection matmuls need wq + the first
        # ht chunk only. Issue wq part 0 and ht tile 0 first, then the rest.
        wq_sb = consts.tile([128, CH, DLOC], BF16)
        wk_sb = consts.tile([128, CH, DLOC], BF16)
        wv_sb = consts.tile([128, CH, DLOC], BF16)
        nc.sync.dma_start(out=wq_sb[:, 0:4, :], in_=wq[:, 0:4, :])
        ht0_a = ht_pool.tile([128, CH // 2, 128], BF16, tag="hta")
        nc.sync.dma_start(out=ht0_a[:], in_=ht[0][:, 0:CH // 2, :])
        ht0_b = ht_pool.tile([128, CH // 2, 128], BF16, tag="htb")
        nc.sync.dma_start(out=ht0_b[:], in_=ht[0][:, CH // 2:, :])
        nc.sync.dma_start(out=wq_sb[:, 4:, :], in_=wq[:, 4:, :])
        nc.sync.dma_start(out=wk_sb[:], in_=wk)
        nc.sync.dma_start(out=wv_sb[:], in_=wv)
        wo_sb = consts.tile([128, HLOC, D], F32R)

        # persistent per-head transposed q/k and v
        qT_sb = persist.tile([128, HLOC, st_n, 128], BF16)
        kT_sb = persist.tile([128, HLOC, st_n, 128], BF16)
        v_sb = persist.tile([128, st_n, DLOC], BF16)

        # ---------------- phase 1: projections + norm + rope + transpose ----
        with tc.tile_pool(name="ps1", bufs=2, space="PSUM") as ps1:
            for st in range(st_n):
                ss = slice(st * 128, (st + 1) * 128)
                if st == 0:
                    ht_a, ht_b = ht0_a, ht0_b
                else:
                    ht_a = ht_pool.tile([128, CH // 2, 128], BF16, tag="hta")
                    nc.sync.dma_start(out=ht_a[:], in_=ht[st][:, 0:CH // 2, :])
                    ht_b = ht_pool.tile([128, CH // 2, 128], BF16, tag="htb")
                    nc.sync.dma_start(out=ht_b[:], in_=ht[st][:, CH // 2:, :])
                cgq_t = cs_pool.tile([128, HD], BF16, tag="cgq")
                nc.sync.dma_start(out=cgq_t[:], in_=cgq[ss, :])
                sgq_t = cs_pool.tile([128, HD], BF16, tag="sgq")
                nc.sync.dma_start(out=sgq_t[:], in_=sgq[ss, :])
                cgk_t = cs_pool.tile([128, HD], BF16, tag="cgk")
                nc.sync.dma_start(out=cgk_t[:], in_=cgk[ss, :])
                sgk_t = cs_pool.tile([128, HD], BF16, tag="sgk")
                nc.sync.dma_start(out=sgk_t[:], in_=sgk[ss, :])

                pq = ps1.tile([128, DLOC], F32, tag="pq")
                pk = ps1.tile([128, DLOC], F32, tag="pk")
                pv = ps1.tile([128, DLOC], F32, tag="pv")
                # proj-major: tile 0's q matmuls only need wq + ht; the
                # q stats/rope chain starts a full projection earlier.
                for psum, wsb in ((pq, wq_sb), (pk, wk_sb), (pv, wv_sb)):
                    for c in range(CH):
                        lhs = (ht_a if c < CH // 2 else ht_b)[:, c % (CH // 2), :]
                        nc.tensor.matmul(psum[:], lhs, wsb[:, c, :],
                                         start=(c == 0), stop=(c == CH - 1))

                # v: PSUM -> SBUF bf16 (ACT is idle in phase 1)
                nc.scalar.copy(v_sb[:, st, :], pv[:])

                # rms stats: Square with free-dim accumulation on ACT (a
                # DVE STT can't read both of its tensor inputs from PSUM)
                varq = work.tile([128, HLOC], F32, tag="varq")
                vark = work.tile([128, HLOC], F32, tag="vark")
                sqd = work.tile([128, HD], F32, tag="sqd")  # dump
                for h in range(HLOC):
                    hs = slice(h * HD, (h + 1) * HD)
                    nc.scalar.activation(sqd[:], pq[:, hs], AF.Square,
                                         accum_out=varq[:, h:h + 1])
                    nc.scalar.activation(sqd[:], pk[:, hs], AF.Square,
                                         accum_out=vark[:, h:h + 1])
                sigq = work.tile([128, HLOC], F32, tag="sigq")
                nc.scalar.activation(sigq[:], varq[:], AF.Sqrt,
                                     bias=eps_t[:], scale=1.0 / HD)
                rq = work.tile([128, HLOC], F32, tag="rq")
                nc.vector.reciprocal_approx_fast(rq[:], sigq[:])
                sigk = work.tile([128, HLOC], F32, tag="sigk")
                nc.scalar.activation(sigk[:], vark[:], AF.Sqrt,
                                     bias=eps_t[:], scale=1.0 / HD)
                rk = work.tile([128, HLOC], F32, tag="rk")
                nc.vector.reciprocal_approx_fast(rk[:], sigk[:])

                # rope, 1/rms fused: out = (r*x).CG + (r*shift64(x)).SG
                for name, psrc, r, cg, sg, dstT in (
                    ("q", pq, rq, cgq_t, sgq_t, qT_sb),
                    ("k", pk, rk, cgk_t, sgk_t, kT_sb),
                ):
                    m1 = work.tile([128, DLOC], F32, tag=f"m1{name}")
                    m2 = work.tile([128, DLOC], F32, tag=f"m2{name}")
                    for h in range(HLOC):
                        hs = slice(h * HD, (h + 1) * HD)
                        lo = slice(h * HD, h * HD + 64)
                        hi = slice(h * HD + 64, (h + 1) * HD)
                        nc.vector.scalar_tensor_tensor(
                            m1[:, hs], psrc[:, hs], r[:, h:h + 1], cg[:],
                            ALU.mult, ALU.mult)
                        nc.vector.scalar_tensor_tensor(
                            m2[:, lo], psrc[:, hi], r[:, h:h + 1], sg[:, 0:64],
                            ALU.mult, ALU.mult)
                        nc.vector.scalar_tensor_tensor(
                            m2[:, hi], psrc[:, lo], r[:, h:h + 1], sg[:, 64:],
                            ALU.mult, ALU.mult)
                    xa = work.tile([128, DLOC], BF16, tag=f"xa{name}")
                    nc.vector.tensor_add(xa[:], m1[:], m2[:])
                    for h in range(HLOC):
                        hs = slice(h * HD, (h + 1) * HD)
                        # DMA-XBAR transpose on the ACT hwdge queue (keeps
                        # the sync queue in plain-copy xbar mode)
                        nc.scalar.dma_start(out=dstT[:, h, st, :],
                                            in_=xa[:, hs], transpose=True)

        # wo is only needed by the drip-fed output projection; its first use
        # is one full q-block into phase 2.
        nc.sync.dma_start(out=wo_sb[:], in_=wo)

        # ---------------- phase 2+3: attention + output projection ----------
        # k-groups alternate between a 3-bank (P) and 2-bank (Q) PSUM score
        # region; exp(g) drains region g while scores(g+1) fill the other.
        # PV matmuls for group g are emitted AFTER scores(g+1) so the
        # in-order PE queue never stalls behind ACT's exp. The denominator
        # runs entirely off the PE: 3-lane bf16 acc on DVE, cross-partition
        # all-reduce on GpSimd, reciprocal + normalize on DVE. The previous
        # q-block's output projection is drip-fed between groups (own PSUM
        # bank, serialized by its DVE drain - never blocks PE on PE-work).
        groups = []
        kt0 = 0
        gi = 0
        while kt0 < kt_n:
            cap = 3 if gi % 2 == 0 else 2
            glen = min(cap, kt_n - kt0)
            groups.append((kt0, glen, gi % 2))
            kt0 += glen
            gi += 1

        with (
            tc.tile_pool(name="psP", bufs=1, space="PSUM") as psP,
            tc.tile_pool(name="psQ", bufs=1, space="PSUM") as psQ,
            tc.tile_pool(name="psO", bufs=1, space="PSUM") as psO,
        ):
            pending = []  # out-proj chunk emitters from the previous q block

            def outproj_chunks(qb, ats):
                chunks = []
                for sti in range(qb_st):
                    st = qb * qb_st + sti
                    sl = slice(sti * 128, (sti + 1) * 128)
                    for nchunk in range(D // 512):
                        ns = slice(nchunk * 512, (nchunk + 1) * 512)

                        def emit(st=st, sl=sl, ns=ns, ats=ats):
                            pf = psO.tile([128, 512], F32, tag="pf", bufs=1,
                                          name=f"pf_{st}_{ns.start}")
                            for h in range(HLOC):
                                nc.tensor.matmul(pf[:], ats[h][:, sl],
                                                 wo_sb[:, h, ns],
                                                 start=(h == 0),
                                                 stop=(h == HLOC - 1))
                            fin = fin_pool.tile([128, 512], F32R, tag="fin",
                                                name=f"fin_{st}_{ns.start}")
                            nc.vector.tensor_copy(fin[:], pf[:])
                            nc.sync.dma_start(
                                out=out[st * 128:(st + 1) * 128, ns],
                                in_=fin[:])
                        chunks.append(emit)
                return chunks

            for qb in range(qb_n):
                ats = []
                for h in range(HLOC):
                    acc = acc_pool.tile([128, 3, QCOLS], BF16, tag="acc")
                    po = psO.tile([128, QCOLS], F32, tag="po", bufs=2,
                                  name=f"po_{qb}_{h}")
                    q_rhs = qT_sb[:, h, qb * qb_st:(qb + 1) * qb_st, :]

                    def flush_pv(et, kt0, glen, po=po, h=h):
                        for j in range(glen):
                            kt = kt0 + j
                            nc.tensor.matmul(po[:],
                                             v_sb[:, kt, h * HD:(h + 1) * HD],
                                             et[:, j, :], start=(kt == 0),
                                             stop=(kt == kt_n - 1))

                    first = True
                    prev = None
                    for (kt0, glen, reg) in groups:
                        pool_ = psP if reg == 0 else psQ
                        width = 3 if reg == 0 else 2
                        sc = pool_.tile([128, width, QCOLS], F32, tag="sc",
                                        name=f"sc{reg}")
                        for j in range(glen):
                            nc.tensor.matmul(sc[:, j, :],
                                             kT_sb[:, h, kt0 + j, :],
                                             q_rhs, start=True, stop=True)
                        et = et_pool.tile([128, 3, QCOLS], BF16, tag="et")
                        nc.scalar.activation(et[:, 0:glen, :],
                                             sc[:, 0:glen, :], AF.Exp)
                        if first:
                            nc.vector.tensor_copy(acc[:, 0:glen, :],
                                                  et[:, 0:glen, :])
                            first = False
                        else:
                            nc.vector.tensor_add(acc[:, 0:glen, :],
                                                 acc[:, 0:glen, :],
                                                 et[:, 0:glen, :])
                        if prev is not None:
                            flush_pv(*prev)
                        prev = (et, kt0, glen)
                        if pending:
                            pending.pop(0)()
                    flush_pv(*prev)

                    # denominator + normalization, no PE involvement
                    fold = den_pool.tile([128, QCOLS], BF16, tag="fold")
                    nc.vector.tensor_add(fold[:], acc[:, 0, :], acc[:, 1, :])
                    nc.vector.tensor_add(fold[:], fold[:], acc[:, 2, :])
                    den = den_pool.tile([128, QCOLS], F32, tag="den")
                    nc.gpsimd.partition_all_reduce(
                        den[:], fold[:], channels=128,
                        reduce_op=bass_isa.ReduceOp.add)
                    rden = den_pool.tile([128, QCOLS], F32, tag="rden")
                    nc.vector.reciprocal_approx_fast(rden[:], den[:])
                    at = at_pool.tile([128, QCOLS], F32R, tag="at")
                    nc.vector.tensor_mul(at[:], po[:], rden[:])
                    ats.append(at)

                pending += outproj_chunks(qb, ats)
            while pending:
                pending.pop(0)()


def build_program(s_len=S):
    nc = bacc.Bacc("TRN2", target_bir_lowering=False, debug=False,
                   enable_asserts=False)
    st_n = s_len // 128
    io = {
        "ht": nc.dram_tensor("ht", [st_n, 128, CH, 128], BF16,
                             kind="ExternalInput").ap(),
        "wq": nc.dram_tensor("wq", [128, CH, DLOC], BF16,
                             kind="ExternalInput").ap(),
        "wk": nc.dram_tensor("wk", [128, CH, DLOC], BF16,
                             kind="ExternalInput").ap(),
        "wv": nc.dram_tensor("wv", [128, CH, DLOC], BF16,
                             kind="ExternalInput").ap(),
        "wo": nc.dram_tensor("wo", [128, HLOC, D], F32R,
                             kind="ExternalInput").ap(),
        "cgq": nc.dram_tensor("cgq", [s_len, HD], BF16,
                              kind="ExternalInput").ap(),
        "sgq": nc.dram_tensor("sgq", [s_len, HD], BF16,
                              kind="ExternalInput").ap(),
        "cgk": nc.dram_tensor("cgk", [s_len, HD], BF16,
                              kind="ExternalInput").ap(),
        "sgk": nc.dram_tensor("sgk", [s_len, HD], BF16,
                              kind="ExternalInput").ap(),
        "out": nc.dram_tensor("out", [s_len, D], F32R,
                              kind="ExternalOutput").ap(),
    }
    with tile.TileContext(nc) as tc:
        build(nc, tc, io, s_len)
    nc.compile()
    return nc


def prep_inputs(inputs, s_len=S):
    """Host-side preprocessing: transposed/tiled bf16 layouts + rope
    coefficient tables (g gains and the 1/sqrt(Hd) scale folded in),
    single head width, bf16."""
    bf16 = ml_dtypes.bfloat16
    hs = np.asarray(inputs["hidden_states"], np.float32).reshape(s_len, D)
    st_n = s_len // 128
    ht = np.ascontiguousarray(
        hs.reshape(st_n, 128, CH, 128).transpose(0, 3, 2, 1)).astype(bf16)

    fc = np.asarray(inputs["freqs_cis"], np.float32).reshape(s_len, HD)
    cos = np.cos(fc)
    sin = np.sin(fc)
    gq = np.asarray(inputs["gq"], np.float32)
    gk = np.asarray(inputs["gk"], np.float32)

    def coef(g, scale):
        cg = cos * g[None, :] * scale
        sg = np.empty_like(sin)
        sg[:, :64] = -sin[:, :64] * g[None, 64:] * scale
        sg[:, 64:] = sin[:, 64:] * g[None, :64] * scale
        return np.ascontiguousarray(cg).astype(bf16), \
            np.ascontiguousarray(sg).astype(bf16)

    cgq, sgq = coef(gq, SCL)
    cgk, sgk = coef(gk, 1.0)

    Wq = np.asarray(inputs["Wq"], np.float32)
    Wk = np.asarray(inputs["Wk"], np.float32)
    Wv = np.asarray(inputs["Wv"], np.float32)
    Wo = np.asarray(inputs["Wo"], np.float32)

    in_maps = []
    for c in range(NCORES):
        cols = slice(DLOC * c, DLOC * (c + 1))
        wq_c = np.ascontiguousarray(
            Wq[cols, :].T.reshape(CH, 128, DLOC).transpose(1, 0, 2)).astype(bf16)
        wk_c = np.ascontiguousarray(
            Wk[cols, :].T.reshape(CH, 128, DLOC).transpose(1, 0, 2)).astype(bf16)
        wv_c = np.ascontiguousarray(
            Wv[cols, :].T.reshape(CH, 128, DLOC).transpose(1, 0, 2)).astype(bf16)
        wo_c = np.ascontiguousarray(
            Wo[:, cols].T.reshape(HLOC, 128, D).transpose(1, 0, 2))
        in_maps.append({
            "ht": ht, "wq": wq_c, "wk": wk_c, "wv": wv_c, "wo": wo_c,
            "cgq": cgq, "sgq": sgq, "cgk": cgk, "sgk": sgk,
        })
    return in_maps


_CACHE = {}


def run_full(inputs, trace=False, **kw):
    if "nc" not in _CACHE:
        _CACHE["nc"] = build_program(S)
    nc = _CACHE["nc"]
    in_maps = prep_inputs(inputs, S)
    res = bass_utils.run_bass_kernel_spmd(
        nc, in_maps, core_ids=list(range(NCORES)), trace=trace, **kw)
    total = res.results[0]["out"].astype(np.float64)
    for c in range(1, NCORES):
        total += res.results[c]["out"]
    total += np.asarray(inputs["bo"], np.float64)[None, :]
    out = total.astype(np.float32).reshape(1, S, D)
    return out, res


def kernel(**inputs):
    out, _ = run_full(inputs, trace=False)
    return out


# revision 8
# speedup vs baseline: 1.5723x; 1.5723x over previous
"""Trainium2 Bass kernel for ErnieImageAttention (non-causal MHA with per-head
RMSNorm on q/k + rotary embedding), tensor-parallel over heads on 8 NeuronCores.

Sharding: 16 heads / 8 cores = 2 heads per core. Each core computes its heads'
q/k/v projections, attention, and a partial output projection (row-parallel
Wo); the host sums the 8 partials and adds the bias.

Per-core dataflow (S=4096, D=2048, Hd=128, 2 local heads):
  phase 1 (per 128-row s-tile):
    q/k/v = hiddenT-chunk matmuls (bf16, N=256 = both heads) accumulated in
    PSUM, emitted proj-major (all q, then k, then v) so tile-0 compute starts
    as soon as Wq lands; RMSNorm stats via ONE fused DVE scalar_tensor_tensor
    (x*x with accum_out) per head; RoPE via host-precomputed bf16 coefficient
    tables (g gains and the 1/sqrt(Hd) logit scale folded in) with the 1/rms
    factor fused into the same DVE ops (scalar operand); q/k transposed to
    [d, s] via DMA-XBAR transposes issued on the ACT hwdge queue (no PE
    transposes, no ACT copies); v kept [s, d] bf16.
  phase 2 (per q-block of 512 cols, per head, k-tiles in groups of 3/2):
    scoresT[k,q] = kT.T @ qT (bf16) into two ping-pong PSUM regions (3+2
    banks); one wide Exp per group on ACT (the phase-2 bottleneck engine);
    group sums accumulated into a 3-lane bf16 acc on DVE; denominator via
    GpSimd partition_all_reduce (no PE work), reciprocal + normalization on
    DVE; attn_T[d,q] = sum_k V[k,d]^T expT[k,q] accumulated in PSUM (po,
    2 banks ping-pong across heads).
  phase 3 (drip-fed between k-groups): fin[s, :2048] = sum_h attnT_h.T @
    WoT_h (f32r) in a single dedicated PSUM bank, drained on DVE, DMA'd out.

Softmax is max-subtraction-free: logits are ~N(0,1) by construction
(RMSNorm'd q/k, 1/sqrt(Hd) folded into q's rope tables).
"""

import numpy as np
import ml_dtypes

import concourse.bass as bass
import concourse.tile as tile
from concourse import bacc, mybir, bass_isa
from concourse import bass_utils
from concourse.masks import make_identity

F32 = mybir.dt.float32
F32R = mybir.dt.float32r
BF16 = mybir.dt.bfloat16
AX = mybir.AxisListType
AF = mybir.ActivationFunctionType
ALU = mybir.AluOpType

S = 4096
D = 2048
HD = 128
HEADS = 16
NCORES = 8
HLOC = HEADS // NCORES  # 2 heads per core
DLOC = HLOC * HD  # 256 local head dims
CH = D // 128  # 16 contraction chunks for projections
EPS = 1e-5
SCL = 1.0 / np.sqrt(HD)

QCOLS = 512  # q columns per attention block


def build(nc, tc, io, s_len):
    st_n = s_len // 128  # s tiles
    qb_n = s_len // QCOLS  # q blocks
    qb_st = QCOLS // 128  # s tiles per q block
    kt_n = st_n  # k tiles

    ht, wq, wk, wv, wo, cgq, sgq, cgk, sgk, out = (
        io["ht"], io["wq"], io["wk"], io["wv"], io["wo"],
        io["cgq"], io["sgq"], io["cgk"], io["sgk"], io["out"],
    )

    import contextlib

    with contextlib.ExitStack() as ctx:
        ctx.enter_context(nc.allow_low_precision(
            reason="bf16/f32r compute; values are O(1) and the rel-err "
                   "budget is 2e-2"))
        consts = ctx.enter_context(tc.tile_pool(name="consts", bufs=1))
        persist = ctx.enter_context(tc.tile_pool(name="persist", bufs=1))
        ht_pool = ctx.enter_context(tc.tile_pool(name="ht", bufs=2))
        cs_pool = ctx.enter_context(tc.tile_pool(name="cs", bufs=2))
        work = ctx.enter_context(tc.tile_pool(name="work", bufs=2))
        et_pool = ctx.enter_context(tc.tile_pool(name="et", bufs=4))
        at_pool = ctx.enter_context(tc.tile_pool(name="at", bufs=6))
        acc_pool = ctx.enter_context(tc.tile_pool(name="acc", bufs=2))
        den_pool = ctx.enter_context(tc.tile_pool(name="den", bufs=2))
        fin_pool = ctx.enter_context(tc.tile_pool(name="fin", bufs=6))

        eps_t = consts.tile([128, 1], F32)
        nc.vector.memset(eps_t[:], EPS)
        ident = consts.tile([128, 128], BF16)
        make_identity(nc, ident[:])

        # Startup ordering: the first q-projection matmuls need wq + the first
        # ht chunk only. Issue wq part 0 and ht tile 0 first, then the rest.
        wq_sb = consts.tile([128, CH, DLOC], BF16)
        wk_sb = consts.tile([128, CH, DLOC], BF16)
        wv_sb = consts.tile([128, CH, DLOC], BF16)
        nc.sync.dma_start(out=wq_sb[:, 0:4, :], in_=wq[:, 0:4, :])
        ht0_a = ht_pool.tile([128, CH // 2, 128], BF16, tag="hta")
        nc.sync.dma_start(out=ht0_a[:], in_=ht[0][:, 0:CH // 2, :])
        ht0_b = ht_pool.tile([128, CH // 2, 128], BF16, tag="htb")
        nc.sync.dma_start(out=ht0_b[:], in_=ht[0][:, CH // 2:, :])
        nc.sync.dma_start(out=wq_sb[:, 4:, :], in_=wq[:, 4:, :])
        nc.sync.dma_start(out=wk_sb[:], in_=wk)
        nc.sync.dma_start(out=wv_sb[:], in_=wv)
        wo_sb = consts.tile([128, HLOC, D], F32R)

        # persistent per-head transposed q/k and v
        qT_sb = persist.tile([128, HLOC, st_n, 128], BF16)
        kT_sb = persist.tile([128, HLOC, st_n, 128], BF16)
        v_sb = persist.tile([128, st_n, DLOC], BF16)

        # ---------------- phase 1: projections + norm + rope + transpose ----
        with tc.tile_pool(name="ps1", bufs=2, space="PSUM") as ps1:
            for st in range(st_n):
                ss = slice(st * 128, (st + 1) * 128)
                if st == 0:
                    ht_a, ht_b = ht0_a, ht0_b
                else:
                    ht_a = ht_pool.tile([128, CH // 2, 128], BF16, tag="hta")
                    nc.sync.dma_start(out=ht_a[:], in_=ht[st][:, 0:CH // 2, :])
                    ht_b = ht_pool.tile([128, CH // 2, 128], BF16, tag="htb")
                    nc.sync.dma_start(out=ht_b[:], in_=ht[st][:, CH // 2:, :])
                cgq_t = cs_pool.tile([128, HD], BF16, tag="cgq")
                nc.sync.dma_start(out=cgq_t[:], in_=cgq[ss, :])
                sgq_t = cs_pool.tile([128, HD], BF16, tag="sgq")
                nc.sync.dma_start(out=sgq_t[:], in_=sgq[ss, :])
                cgk_t = cs_pool.tile([128, HD], BF16, tag="cgk")
                nc.sync.dma_start(out=cgk_t[:], in_=cgk[ss, :])
                sgk_t = cs_pool.tile([128, HD], BF16, tag="sgk")
                nc.sync.dma_start(out=sgk_t[:], in_=sgk[ss, :])

                pq = ps1.tile([128, DLOC], F32, tag="pq")
                pk = ps1.tile([128, DLOC], F32, tag="pk")
                pv = ps1.tile([128, DLOC], F32, tag="pv")
                # proj-major: tile 0's q matmuls only need wq + ht; the
                # q stats/rope chain starts a full projection earlier.
                for psum, wsb in ((pq, wq_sb), (pk, wk_sb), (pv, wv_sb)):
                    for c in range(CH):
                        lhs = (ht_a if c < CH // 2 else ht_b)[:, c % (CH // 2), :]
                        nc.tensor.matmul(psum[:], lhs, wsb[:, c, :],
                                         start=(c == 0), stop=(c == CH - 1))

                # v: PSUM -> SBUF bf16 (ACT is idle in phase 1)
                nc.scalar.copy(v_sb[:, st, :], pv[:])

                # rms stats: Square with free-dim accumulation on ACT (a
                # DVE STT can't read both of its tensor inputs from PSUM)
                varq = work.tile([128, HLOC], F32, tag="varq")
                vark = work.tile([128, HLOC], F32, tag="vark")
                sqd = work.tile([128, HD], F32, tag="sqd")  # dump
                for h in range(HLOC):
                    hs = slice(h * HD, (h + 1) * HD)
                    nc.scalar.activation(sqd[:], pq[:, hs], AF.Square,
                                         accum_out=varq[:, h:h + 1])
                    nc.scalar.activation(sqd[:], pk[:, hs], AF.Square,
                                         accum_out=vark[:, h:h + 1])
                sigq = work.tile([128, HLOC], F32, tag="sigq")
                nc.scalar.activation(sigq[:], varq[:], AF.Sqrt,
                                     bias=eps_t[:], scale=1.0 / HD)
                rq = work.tile([128, HLOC], F32, tag="rq")
                nc.vector.reciprocal_approx_fast(rq[:], sigq[:])
                sigk = work.tile([128, HLOC], F32, tag="sigk")
                nc.scalar.activation(sigk[:], vark[:], AF.Sqrt,
                                     bias=eps_t[:], scale=1.0 / HD)
                rk = work.tile([128, HLOC], F32, tag="rk")
                nc.vector.reciprocal_approx_fast(rk[:], sigk[:])

                # rope, 1/rms fused: out = (r*x).CG + (r*shift64(x)).SG
                for name, psrc, r, cg, sg, dstT in (
                    ("q", pq, rq, cgq_t, sgq_t, qT_sb),
                    ("k", pk, rk, cgk_t, sgk_t, kT_sb),
                ):
                    m1 = work.tile([128, DLOC], F32, tag=f"m1{name}")
                    m2 = work.tile([128, DLOC], F32, tag=f"m2{name}")
                    for h in range(HLOC):
                        hs = slice(h * HD, (h + 1) * HD)
                        lo = slice(h * HD, h * HD + 64)
                        hi = slice(h * HD + 64, (h + 1) * HD)
                        nc.vector.scalar_tensor_tensor(
                            m1[:, hs], psrc[:, hs], r[:, h:h + 1], cg[:],
                            ALU.mult, ALU.mult)
                        nc.vector.scalar_tensor_tensor(
                            m2[:, lo], psrc[:, hi], r[:, h:h + 1], sg[:, 0:64],
                            ALU.mult, ALU.mult)
                        nc.vector.scalar_tensor_tensor(
                            m2[:, hi], psrc[:, lo], r[:, h:h + 1], sg[:, 64:],
                            ALU.mult, ALU.mult)
                    xa = work.tile([128, DLOC], BF16, tag=f"xa{name}")
                    nc.vector.tensor_add(xa[:], m1[:], m2[:])
                    for h in range(HLOC):
                        hs = slice(h * HD, (h + 1) * HD)
                        ptp = ps1.tile([128, 128], BF16, tag="ptp")
                        nc.tensor.transpose(ptp[:], xa[:, hs], ident[:])
                        nc.vector.tensor_copy(dstT[:, h, st, :], ptp[:])

        # wo is only needed by the drip-fed output projection; its first use
        # is one full q-block into phase 2.
        nc.sync.dma_start(out=wo_sb[:], in_=wo)

        # ---------------- phase 2+3: attention + output projection ----------
        # k-groups alternate between a 3-bank (P) and 2-bank (Q) PSUM score
        # region; exp(g) drains region g while scores(g+1) fill the other.
        # PV matmuls for group g are emitted AFTER scores(g+1) so the
        # in-order PE queue never stalls behind ACT's exp. The denominator
        # runs entirely off the PE: 3-lane bf16 acc on DVE, cross-partition
        # all-reduce on GpSimd, reciprocal + normalize on DVE. The previous
        # q-block's output projection is drip-fed between groups (own PSUM
        # bank, serialized by its DVE drain - never blocks PE on PE-work).
        groups = []
        kt0 = 0
        gi = 0
        while kt0 < kt_n:
            cap = 3 if gi % 2 == 0 else 2
            glen = min(cap, kt_n - kt0)
            groups.append((kt0, glen, gi % 2))
            kt0 += glen
            gi += 1

        with (
            tc.tile_pool(name="psP", bufs=1, space="PSUM") as psP,
            tc.tile_pool(name="psQ", bufs=1, space="PSUM") as psQ,
            tc.tile_pool(name="psO", bufs=1, space="PSUM") as psO,
        ):
            pending = []  # deferred emitters (at-muls + out-proj chunks)

            def outproj_chunks(qb, ats, tags=("pf",)):
                chunks = []
                for i, (sti, nchunk) in enumerate(
                        (a, b) for a in range(qb_st) for b in range(D // 512)):
                    st = qb * qb_st + sti
                    sl = slice(sti * 128, (sti + 1) * 128)
                    ns = slice(nchunk * 512, (nchunk + 1) * 512)
                    tg = tags[i % len(tags)]

                    def emit(st=st, sl=sl, ns=ns, ats=ats, tg=tg):
                        pf = psO.tile([128, 512], F32, tag=tg,
                                      bufs=(2 if tg == "po" else 1),
                                      name=f"pf_{st}_{ns.start}")
                        for h in range(HLOC):
                            nc.tensor.matmul(pf[:], ats[h][:, sl],
                                             wo_sb[:, h, ns],
                                             start=(h == 0),
                                             stop=(h == HLOC - 1))
                        fin = fin_pool.tile([128, 512], F32R, tag="fin",
                                            name=f"fin_{st}_{ns.start}")
                        nc.vector.tensor_copy(fin[:], pf[:])
                        nc.sync.dma_start(
                            out=out[st * 128:(st + 1) * 128, ns],
                            in_=fin[:])
                    chunks.append(emit)
                return chunks

            # One continuous software-pipelined stream over (qb, head, group):
            # scores(i) -> exp(i) -> acc(i) -> PV(i-1) -> one deferred pop.
            # The pipeline never drains at head/q-block boundaries; tails
            # (fold/all-reduce/recip) are DVE/GpSimd-only and emit inline,
            # while the po-consuming at-mul is deferred via `pending` so the
            # DVE queue never blocks on the GpSimd latency.
            tasks = []
            for qb in range(qb_n):
                for h in range(HLOC):
                    for g_idx, (kt0, glen, reg) in enumerate(groups):
                        tasks.append((qb, h, g_idx, kt0, glen, reg))

            prev = None  # (et, kt0, glen, h, po)
            acc = None
            po = None
            ats = {}
            for (qb, h, g_idx, kt0, glen, reg) in tasks:
                if g_idx == 0:
                    acc = acc_pool.tile([128, 3, QCOLS], BF16, tag="acc",
                                        name=f"acc_{qb}_{h}")
                    po = psO.tile([128, QCOLS], F32, tag="po", bufs=2,
                                  name=f"po_{qb}_{h}")
                q_rhs = qT_sb[:, h, qb * qb_st:(qb + 1) * qb_st, :]
                pool_ = psP if reg == 0 else psQ
                width = 3 if reg == 0 else 2
                sc = pool_.tile([128, width, QCOLS], F32, tag="sc",
                                name=f"sc{reg}")
                for j in range(glen):
                    nc.tensor.matmul(sc[:, j, :], kT_sb[:, h, kt0 + j, :],
                                     q_rhs, start=True, stop=True)
                et = et_pool.tile([128, 3, QCOLS], BF16, tag="et")
                nc.scalar.activation(et[:, 0:glen, :], sc[:, 0:glen, :],
                                     AF.Exp)
                if g_idx == 0:
                    nc.vector.tensor_copy(acc[:, 0:glen, :], et[:, 0:glen, :])
                else:
                    nc.vector.tensor_add(acc[:, 0:glen, :],
                                         acc[:, 0:glen, :], et[:, 0:glen, :])
                if prev is not None:
                    pet, pkt0, pglen, ph, ppo = prev
                    for j in range(pglen):
                        kt = pkt0 + j
                        nc.tensor.matmul(ppo[:],
                                         v_sb[:, kt, ph * HD:(ph + 1) * HD],
                                         pet[:, j, :], start=(kt == 0),
                                         stop=(kt == kt_n - 1))
                prev = (et, kt0, glen, h, po)
                if len(pending) > 2:
                    pending.pop(0)()

                if g_idx == len(groups) - 1:
                    # head tail: denominator off the PE, at-mul deferred
                    fold = den_pool.tile([128, QCOLS], BF16, tag="fold")
                    nc.vector.tensor_add(fold[:], acc[:, 0, :], acc[:, 1, :])
                    nc.vector.tensor_add(fold[:], fold[:], acc[:, 2, :])
                    den = den_pool.tile([128, QCOLS], F32, tag="den")
                    nc.gpsimd.partition_all_reduce(
                        den[:], fold[:], channels=128,
                        reduce_op=bass_isa.ReduceOp.add)
                    rden = den_pool.tile([128, QCOLS], F32, tag="rden")
                    nc.vector.reciprocal_approx_fast(rden[:], den[:])
                    at = at_pool.tile([128, QCOLS], F32R, tag="at",
                                      name=f"at_{qb}_{h}")
                    ats[(qb, h)] = at

                    def at_mul(at=at, po=po, rden=rden):
                        nc.vector.tensor_mul(at[:], po[:], rden[:])
                    pending.append(at_mul)
                    if h == HLOC - 1:
                        pending += outproj_chunks(
                            qb, [ats[(qb, 0)], ats[(qb, 1)]])

            # drain: last PV flush, then remaining pops with the po banks
            # (now free) joining the pf bank in a ping-pong
            pet, pkt0, pglen, ph, ppo = prev
            for j in range(pglen):
                kt = pkt0 + j
                nc.tensor.matmul(ppo[:], v_sb[:, kt, ph * HD:(ph + 1) * HD],
                                 pet[:, j, :], start=(kt == 0),
                                 stop=(kt == kt_n - 1))
            # the last head's at-mul and chunks are still queued; re-emit the
            # last q-block's chunks alternating pf/po tags for the tail
            tail_chunks = []
            while pending:
                item = pending.pop(0)
                tail_chunks.append(item)
            # first items may be at-muls (emit as-is); chunk emitters for the
            # final q-block get replaced by tag-alternating versions
            n_final = min(len(tail_chunks), D // 512 * qb_st)
            for item in tail_chunks[:-n_final] if n_final else tail_chunks:
                item()
            if n_final:
                alt = outproj_chunks(qb_n - 1,
                                     [ats[(qb_n - 1, 0)], ats[(qb_n - 1, 1)]],
                                     tags=("pf", "po"))
                # only re-emit the ones not already emitted
                for item in alt[len(alt) - n_final:]:
                    item()


def build_program(s_len=S):
    nc = bacc.Bacc("TRN2", target_bir_lowering=False, debug=False,
                   enable_asserts=False)
    st_n = s_len // 128
    io = {
        "ht": nc.dram_tensor("ht", [st_n, 128, CH, 128], BF16,
                             kind="ExternalInput").ap(),
        "wq": nc.dram_tensor("wq", [128, CH, DLOC], BF16,
                             kind="ExternalInput").ap(),
        "wk": nc.dram_tensor("wk", [128, CH, DLOC], BF16,
                             kind="ExternalInput").ap(),
        "wv": nc.dram_tensor("wv", [128, CH, DLOC], BF16,
                             kind="ExternalInput").ap(),
        "wo": nc.dram_tensor("wo", [128, HLOC, D], F32R,
                             kind="ExternalInput").ap(),
        "cgq": nc.dram_tensor("cgq", [s_len, HD], BF16,
                              kind="ExternalInput").ap(),
        "sgq": nc.dram_tensor("sgq", [s_len, HD], BF16,
                              kind="ExternalInput").ap(),
        "cgk": nc.dram_tensor("cgk", [s_len, HD], BF16,
                              kind="ExternalInput").ap(),
        "sgk": nc.dram_tensor("sgk", [s_len, HD], BF16,
                              kind="ExternalInput").ap(),
        "out": nc.dram_tensor("out", [s_len, D], F32R,
                              kind="ExternalOutput").ap(),
    }
    with tile.TileContext(nc) as tc:
        build(nc, tc, io, s_len)
    nc.compile()
    return nc


def prep_inputs(inputs, s_len=S):
    """Host-side preprocessing: transposed/tiled bf16 layouts + rope
    coefficient tables (g gains and the 1/sqrt(Hd) scale folded in),
    single head width, bf16."""
    bf16 = ml_dtypes.bfloat16
    hs = np.asarray(inputs["hidden_states"], np.float32).reshape(s_len, D)
    st_n = s_len // 128
    ht = np.ascontiguousarray(
        hs.reshape(st_n, 128, CH, 128).transpose(0, 3, 2, 1)).astype(bf16)

    fc = np.asarray(inputs["freqs_cis"], np.float32).reshape(s_len, HD)
    cos = np.cos(fc)
    sin = np.sin(fc)
    gq = np.asarray(inputs["gq"], np.float32)
    gk = np.asarray(inputs["gk"], np.float32)

    def coef(g, scale):
        cg = cos * g[None, :] * scale
        sg = np.empty_like(sin)
        sg[:, :64] = -sin[:, :64] * g[None, 64:] * scale
        sg[:, 64:] = sin[:, 64:] * g[None, :64] * scale
        return np.ascontiguousarray(cg).astype(bf16), \
            np.ascontiguousarray(sg).astype(bf16)

    cgq, sgq = coef(gq, SCL)
    cgk, sgk = coef(gk, 1.0)

    Wq = np.asarray(inputs["Wq"], np.float32)
    Wk = np.asarray(inputs["Wk"], np.float32)
    Wv = np.asarray(inputs["Wv"], np.float32)
    Wo = np.asarray(inputs["Wo"], np.float32)

    in_maps = []
    for c in range(NCORES):
        cols = slice(DLOC * c, DLOC * (c + 1))
        wq_c = np.ascontiguousarray(
            Wq[cols, :].T.reshape(CH, 128, DLOC).transpose(1, 0, 2)).astype(bf16)
        wk_c = np.ascontiguousarray(
            Wk[cols, :].T.reshape(CH, 128, DLOC).transpose(1, 0, 2)).astype(bf16)
        wv_c = np.ascontiguousarray(
            Wv[cols, :].T.reshape(CH, 128, DLOC).transpose(1, 0, 2)).astype(bf16)
        wo_c = np.ascontiguousarray(
            Wo[:, cols].T.reshape(HLOC, 128, D).transpose(1, 0, 2))
        in_maps.append({
            "ht": ht, "wq": wq_c, "wk": wk_c, "wv": wv_c, "wo": wo_c,
            "cgq": cgq, "sgq": sgq, "cgk": cgk, "sgk": sgk,
        })
    return in_maps


_CACHE = {}


def run_full(inputs, trace=False, **kw):
    if "nc" not in _CACHE:
        _CACHE["nc"] = build_program(S)
    nc = _CACHE["nc"]
    in_maps = prep_inputs(inputs, S)
    res = bass_utils.run_bass_kernel_spmd(
        nc, in_maps, core_ids=list(range(NCORES)), trace=trace, **kw)
    total = res.results[0]["out"].astype(np.float64)
    for c in range(1, NCORES):
        total += res.results[c]["out"]
    total += np.asarray(inputs["bo"], np.float64)[None, :]
    out = total.astype(np.float32).reshape(1, S, D)
    return out, res


def kernel(**inputs):
    out, _ = run_full(inputs, trace=False)
    return out


# revision 14
# speedup vs baseline: 1.5848x; 1.0079x over previous
"""Trainium2 Bass kernel for ErnieImageAttention (non-causal MHA with per-head
RMSNorm on q/k + rotary embedding), tensor-parallel over heads on 8 NeuronCores.

Sharding: 16 heads / 8 cores = 2 heads per core. Each core computes its heads'
q/k/v projections, attention, and a partial output projection (row-parallel
Wo); the host sums the 8 partials and adds the bias.

Per-core dataflow (S=4096, D=2048, Hd=128, 2 local heads):
  phase 1 (per 128-row s-tile):
    q/k/v = hiddenT-chunk matmuls (bf16, N=256 = both heads) accumulated in
    PSUM, emitted proj-major (all q, then k, then v) so tile-0 compute starts
    as soon as Wq lands; RMSNorm stats via ONE fused DVE scalar_tensor_tensor
    (x*x with accum_out) per head; RoPE via host-precomputed bf16 coefficient
    tables (g gains and the 1/sqrt(Hd) logit scale folded in) with the 1/rms
    factor fused into the same DVE ops (scalar operand); q/k transposed to
    [d, s] via DMA-XBAR transposes issued on the ACT hwdge queue (no PE
    transposes, no ACT copies); v kept [s, d] bf16.
  phase 2 (per q-block of 512 cols, per head, k-tiles in groups of 3/2):
    scoresT[k,q] = kT.T @ qT (bf16) into two ping-pong PSUM regions (3+2
    banks); one wide Exp per group on ACT (the phase-2 bottleneck engine);
    group sums accumulated into a 3-lane bf16 acc on DVE; denominator via
    GpSimd partition_all_reduce (no PE work), reciprocal + normalization on
    DVE; attn_T[d,q] = sum_k V[k,d]^T expT[k,q] accumulated in PSUM (po,
    2 banks ping-pong across heads).
  phase 3 (drip-fed between k-groups): fin[s, :2048] = sum_h attnT_h.T @
    WoT_h (f32r) in a single dedicated PSUM bank, drained on DVE, DMA'd out.

Softmax is max-subtraction-free: logits are ~N(0,1) by construction
(RMSNorm'd q/k, 1/sqrt(Hd) folded into q's rope tables).
"""

import numpy as np
import ml_dtypes

import concourse.bass as bass
import concourse.tile as tile
from concourse import bacc, mybir, bass_isa
from concourse import bass_utils
from concourse.masks import make_identity

F32 = mybir.dt.float32
F32R = mybir.dt.float32r
BF16 = mybir.dt.bfloat16
AX = mybir.AxisListType
AF = mybir.ActivationFunctionType
ALU = mybir.AluOpType

S = 4096
D = 2048
HD = 128
HEADS = 16
NCORES = 8
HLOC = HEADS // NCORES  # 2 heads per core
DLOC = HLOC * HD  # 256 local head dims
CH = D // 128  # 16 contraction chunks for projections
EPS = 1e-5
SCL = 1.0 / np.sqrt(HD)

QCOLS = 512  # q columns per attention block


def build(nc, tc, io, s_len):
    st_n = s_len // 128  # s tiles
    qb_n = s_len // QCOLS  # q blocks
    qb_st = QCOLS // 128  # s tiles per q block
    kt_n = st_n  # k tiles

    ht, wq, wk, wv, wo, cgq, sgq, cgk, sgk, out = (
        io["ht"], io["wq"], io["wk"], io["wv"], io["wo"],
        io["cgq"], io["sgq"], io["cgk"], io["sgk"], io["out"],
    )

    import contextlib

    with contextlib.ExitStack() as ctx:
        ctx.enter_context(nc.allow_low_precision(
            reason="bf16/f32r compute; values are O(1) and the rel-err "
                   "budget is 2e-2"))
        consts = ctx.enter_context(tc.tile_pool(name="consts", bufs=1))
        persist = ctx.enter_context(tc.tile_pool(name="persist", bufs=1))
        ht_pool = ctx.enter_context(tc.tile_pool(name="ht", bufs=2))
        cs_pool = ctx.enter_context(tc.tile_pool(name="cs", bufs=2))
        work = ctx.enter_context(tc.tile_pool(name="work", bufs=2))
        et_pool = ctx.enter_context(tc.tile_pool(name="et", bufs=4))
        at_pool = ctx.enter_context(tc.tile_pool(name="at", bufs=6))
        acc_pool = ctx.enter_context(tc.tile_pool(name="acc", bufs=2))
        den_pool = ctx.enter_context(tc.tile_pool(name="den", bufs=2))
        fin_pool = ctx.enter_context(tc.tile_pool(name="fin", bufs=6))

        eps_t = consts.tile([128, 1], F32)
        nc.vector.memset(eps_t[:], EPS)
        ident = consts.tile([128, 128], BF16)
        make_identity(nc, ident[:])

        # Startup ordering: tile 0's data first (ht + rope tables), then the
        # weights part-major so tile 0's part-interleaved matmuls chase the
        # DMA stream with minimal lag.
        ht0_a = ht_pool.tile([128, CH // 2, 128], BF16, tag="hta")
        nc.sync.dma_start(out=ht0_a[:], in_=ht[0][:, 0:CH // 2, :])
        ht0_b = ht_pool.tile([128, CH // 2, 128], BF16, tag="htb")
        nc.sync.dma_start(out=ht0_b[:], in_=ht[0][:, CH // 2:, :])
        s0 = slice(0, 128)
        tab0 = []
        for nm, dram in (("cgq", cgq), ("sgq", sgq), ("cgk", cgk),
                         ("sgk", sgk)):
            t = cs_pool.tile([128, DLOC], BF16, tag=nm, name=f"{nm}0")
            nc.sync.dma_start(out=t[:], in_=dram[s0, :])
            tab0.append(t)
        wq_sb = consts.tile([128, CH, DLOC], BF16)
        wk_sb = consts.tile([128, CH, DLOC], BF16)
        wv_sb = consts.tile([128, CH, DLOC], BF16)
        for p in range(4):
            ps = slice(4 * p, 4 * (p + 1))
            nc.sync.dma_start(out=wq_sb[:, ps, :], in_=wq[:, ps, :])
            nc.sync.dma_start(out=wk_sb[:, ps, :], in_=wk[:, ps, :])
            nc.sync.dma_start(out=wv_sb[:, ps, :], in_=wv[:, ps, :])
        wo_sb = consts.tile([128, HLOC, D], F32R)

        # persistent per-head transposed q/k and v
        qT_sb = persist.tile([128, HLOC, st_n, 128], BF16)
        kT_sb = persist.tile([128, HLOC, st_n, 128], BF16)
        v_sb = persist.tile([128, st_n, DLOC], BF16)

        # ---------------- phase 1: projections + norm + rope + transpose ----
        with tc.tile_pool(name="ps1", bufs=2, space="PSUM") as ps1:
            for st in range(st_n):
                ss = slice(st * 128, (st + 1) * 128)
                if st == 0:
                    ht_a, ht_b = ht0_a, ht0_b
                    cgq_t, sgq_t, cgk_t, sgk_t = tab0
                else:
                    ht_a = ht_pool.tile([128, CH // 2, 128], BF16, tag="hta")
                    nc.sync.dma_start(out=ht_a[:], in_=ht[st][:, 0:CH // 2, :])
                    ht_b = ht_pool.tile([128, CH // 2, 128], BF16, tag="htb")
                    nc.sync.dma_start(out=ht_b[:], in_=ht[st][:, CH // 2:, :])
                    cgq_t = cs_pool.tile([128, DLOC], BF16, tag="cgq")
                    nc.sync.dma_start(out=cgq_t[:], in_=cgq[ss, :])
                    sgq_t = cs_pool.tile([128, DLOC], BF16, tag="sgq")
                    nc.sync.dma_start(out=sgq_t[:], in_=sgq[ss, :])
                    cgk_t = cs_pool.tile([128, DLOC], BF16, tag="cgk")
                    nc.sync.dma_start(out=cgk_t[:], in_=cgk[ss, :])
                    sgk_t = cs_pool.tile([128, DLOC], BF16, tag="sgk")
                    nc.sync.dma_start(out=sgk_t[:], in_=sgk[ss, :])

                pq = ps1.tile([128, DLOC], F32, tag="pq")
                pk = ps1.tile([128, DLOC], F32, tag="pk")
                pv = ps1.tile([128, DLOC], F32, tag="pv")
                # tile 0: part-major interleaved, chasing the weight DMAs;
                # later tiles: proj-major (q first so its stats/rope chain
                # starts a full projection earlier)
                if st == 0:
                    for p in range(4):
                        for psum, wsb in ((pq, wq_sb), (pk, wk_sb),
                                          (pv, wv_sb)):
                            for c in range(4 * p, 4 * (p + 1)):
                                lhs = (ht_a if c < CH // 2
                                       else ht_b)[:, c % (CH // 2), :]
                                nc.tensor.matmul(psum[:], lhs, wsb[:, c, :],
                                                 start=(c == 0),
                                                 stop=(c == CH - 1))
                else:
                    for psum, wsb in ((pq, wq_sb), (pk, wk_sb), (pv, wv_sb)):
                        for c in range(CH):
                            lhs = (ht_a if c < CH // 2
                                   else ht_b)[:, c % (CH // 2), :]
                            nc.tensor.matmul(psum[:], lhs, wsb[:, c, :],
                                             start=(c == 0), stop=(c == CH - 1))

                # v: PSUM -> SBUF bf16 (ACT is idle in phase 1)
                nc.scalar.copy(v_sb[:, st, :], pv[:])

                # rms stats: Square with free-dim accumulation on ACT (a
                # DVE STT can't read both of its tensor inputs from PSUM)
                varq = work.tile([128, HLOC], F32, tag="varq")
                vark = work.tile([128, HLOC], F32, tag="vark")
                sqd = work.tile([128, HD], F32, tag="sqd")  # dump
                for h in range(HLOC):
                    hs = slice(h * HD, (h + 1) * HD)
                    nc.scalar.activation(sqd[:], pq[:, hs], AF.Square,
                                         accum_out=varq[:, h:h + 1])
                    nc.scalar.activation(sqd[:], pk[:, hs], AF.Square,
                                         accum_out=vark[:, h:h + 1])
                sigq = work.tile([128, HLOC], F32, tag="sigq")
                nc.scalar.activation(sigq[:], varq[:], AF.Sqrt,
                                     bias=eps_t[:], scale=1.0 / HD)
                rq = work.tile([128, HLOC], F32, tag="rq")
                nc.vector.reciprocal_approx_fast(rq[:], sigq[:])
                sigk = work.tile([128, HLOC], F32, tag="sigk")
                nc.scalar.activation(sigk[:], vark[:], AF.Sqrt,
                                     bias=eps_t[:], scale=1.0 / HD)
                rk = work.tile([128, HLOC], F32, tag="rk")
                nc.vector.reciprocal_approx_fast(rk[:], sigk[:])

                # rope: out = (r*x) . CG + shift64(r*x) . SG   (per tensor)
                for name, psrc, r, cg, sg, dstT in (
                    ("q", pq, rq, cgq_t, sgq_t, qT_sb),
                    ("k", pk, rk, cgk_t, sgk_t, kT_sb),
                ):
                    xs = work.tile([128, DLOC], F32, tag=f"xs{name}")
                    for h in range(HLOC):
                        hs = slice(h * HD, (h + 1) * HD)
                        nc.vector.tensor_scalar_mul(xs[:, hs], psrc[:, hs],
                                                    r[:, h:h + 1])
                    m1 = work.tile([128, DLOC], F32, tag=f"m1{name}")
                    nc.vector.tensor_mul(m1[:], xs[:], cg[:])
                    m2 = work.tile([128, DLOC], F32, tag=f"m2{name}")
                    x4 = xs[:].rearrange("p (h t u) -> p h t u", h=HLOC, t=2)
                    m4 = m2[:].rearrange("p (h t u) -> p h t u", h=HLOC, t=2)
                    g4 = sg[:].rearrange("p (h t u) -> p h t u", h=HLOC, t=2)
                    nc.vector.tensor_mul(m4[:, :, 0, :], x4[:, :, 1, :],
                                         g4[:, :, 0, :])
                    nc.vector.tensor_mul(m4[:, :, 1, :], x4[:, :, 0, :],
                                         g4[:, :, 1, :])
                    xa = work.tile([128, DLOC], BF16, tag=f"xa{name}")
                    nc.vector.tensor_add(xa[:], m1[:], m2[:])
                    for h in range(HLOC):
                        hs = slice(h * HD, (h + 1) * HD)
                        ptp = ps1.tile([128, 128], BF16, tag="ptp")
                        nc.tensor.transpose(ptp[:], xa[:, hs], ident[:])
                        nc.vector.tensor_copy(dstT[:, h, st, :], ptp[:])

        # wo is only needed by the drip-fed output projection; its first use
        # is one full q-block into phase 2.
        nc.sync.dma_start(out=wo_sb[:], in_=wo)

        # ---------------- phase 2+3: attention + output projection ----------
        # k-groups alternate between a 3-bank (P) and 2-bank (Q) PSUM score
        # region; exp(g) drains region g while scores(g+1) fill the other.
        # PV matmuls for group g are emitted AFTER scores(g+1) so the
        # in-order PE queue never stalls behind ACT's exp. The denominator
        # runs entirely off the PE: 3-lane bf16 acc on DVE, cross-partition
        # all-reduce on GpSimd, reciprocal + normalize on DVE. The previous
        # q-block's output projection is drip-fed between groups (own PSUM
        # bank, serialized by its DVE drain - never blocks PE on PE-work).
        groups = []
        kt0 = 0
        gi = 0
        while kt0 < kt_n:
            cap = 3 if gi % 2 == 0 else 2
            glen = min(cap, kt_n - kt0)
            groups.append((kt0, glen, gi % 2))
            kt0 += glen
            gi += 1

        with (
            tc.tile_pool(name="psP", bufs=1, space="PSUM") as psP,
            tc.tile_pool(name="psQ", bufs=1, space="PSUM") as psQ,
            tc.tile_pool(name="psO", bufs=1, space="PSUM") as psO,
        ):
            pending = []  # deferred emitters (at-muls + out-proj chunks)

            def outproj_chunks(qb, ats, tags=("pf",)):
                chunks = []
                for i, (sti, nchunk) in enumerate(
                        (a, b) for a in range(qb_st) for b in range(D // 512)):
                    st = qb * qb_st + sti
                    sl = slice(sti * 128, (sti + 1) * 128)
                    ns = slice(nchunk * 512, (nchunk + 1) * 512)
                    tg = tags[i % len(tags)]

                    def emit(st=st, sl=sl, ns=ns, ats=ats, tg=tg):
                        pf = psO.tile([128, 512], F32, tag=tg,
                                      bufs=(2 if tg == "po" else 1),
                                      name=f"pf_{st}_{ns.start}")
                        for h in range(HLOC):
                            nc.tensor.matmul(pf[:], ats[h][:, sl],
                                             wo_sb[:, h, ns],
                                             start=(h == 0),
                                             stop=(h == HLOC - 1))
                        fin = fin_pool.tile([128, 512], F32R, tag="fin",
                                            name=f"fin_{st}_{ns.start}")
                        nc.vector.tensor_copy(fin[:], pf[:])
                        nc.sync.dma_start(
                            out=out[st * 128:(st + 1) * 128, ns],
                            in_=fin[:])
                    chunks.append(emit)
                return chunks

            # One continuous software-pipelined stream over (qb, head, group):
            # scores(i) -> exp(i) -> acc(i) -> PV(i-1) -> one deferred pop.
            # The pipeline never drains at head/q-block boundaries; tails
            # (fold/all-reduce/recip) are DVE/GpSimd-only and emit inline,
            # while the po-consuming at-mul is deferred via `pending` so the
            # DVE queue never blocks on the GpSimd latency.
            tasks = []
            for qb in range(qb_n):
                for h in range(HLOC):
                    for g_idx, (kt0, glen, reg) in enumerate(groups):
                        tasks.append((qb, h, g_idx, kt0, glen, reg))

            def pump(idx, budget=2):
                n = 0
                while pending and n < budget and pending[0][0] <= idx:
                    pending.pop(0)[1]()
                    n += 1

            prev = None  # (et, kt0, glen, h, po)
            acc = None
            po = None
            ats = {}
            for ti, (qb, h, g_idx, kt0, glen, reg) in enumerate(tasks):
                if g_idx == 0:
                    acc = acc_pool.tile([128, 3, QCOLS], BF16, tag="acc",
                                        name=f"acc_{qb}_{h}")
                    po = psO.tile([128, QCOLS], F32, tag="po", bufs=2,
                                  name=f"po_{qb}_{h}")
                q_rhs = qT_sb[:, h, qb * qb_st:(qb + 1) * qb_st, :]
                pool_ = psP if reg == 0 else psQ
                width = 3 if reg == 0 else 2
                sc = pool_.tile([128, width, QCOLS], F32, tag="sc",
                                name=f"sc{reg}")
                for j in range(glen):
                    nc.tensor.matmul(sc[:, j, :], kT_sb[:, h, kt0 + j, :],
                                     q_rhs, start=True, stop=True)
                et = et_pool.tile([128, 3, QCOLS], BF16, tag="et")
                nc.scalar.activation(et[:, 0:glen, :], sc[:, 0:glen, :],
                                     AF.Exp)
                if g_idx == 0:
                    nc.vector.tensor_copy(acc[:, 0:glen, :], et[:, 0:glen, :])
                else:
                    nc.vector.tensor_add(acc[:, 0:glen, :],
                                         acc[:, 0:glen, :], et[:, 0:glen, :])
                if prev is not None:
                    pet, pkt0, pglen, ph, ppo = prev
                    for j in range(pglen):
                        kt = pkt0 + j
                        nc.tensor.matmul(ppo[:],
                                         v_sb[:, kt, ph * HD:(ph + 1) * HD],
                                         pet[:, j, :], start=(kt == 0),
                                         stop=(kt == kt_n - 1))
                prev = (et, kt0, glen, h, po)
                pump(ti)

                if g_idx == len(groups) - 1:
                    # head tail: denominator off the PE (DVE fold + GpSimd
                    # all-reduce + DVE reciprocal); the po-consuming at-mul
                    # is deferred past the GpSimd latency so the DVE queue
                    # never blocks on it
                    fold = den_pool.tile([128, QCOLS], BF16, tag="fold")
                    nc.vector.tensor_add(fold[:], acc[:, 0, :], acc[:, 1, :])
                    nc.vector.tensor_add(fold[:], fold[:], acc[:, 2, :])
                    den = den_pool.tile([128, QCOLS], F32, tag="den")
                    nc.gpsimd.partition_all_reduce(
                        den[:], fold[:], channels=128,
                        reduce_op=bass_isa.ReduceOp.add)
                    rden = den_pool.tile([128, QCOLS], F32, tag="rden")
                    nc.vector.reciprocal_approx_fast(rden[:], den[:])
                    at = at_pool.tile([128, QCOLS], F32R, tag="at",
                                      name=f"at_{qb}_{h}")
                    ats[(qb, h)] = at

                    def at_mul(at=at, po=po, rden=rden):
                        nc.vector.tensor_mul(at[:], po[:], rden[:])
                    pending.append((ti + 4, at_mul))
                    if h == HLOC - 1:
                        tags = ("pf", "po") if qb == qb_n - 1 else ("pf",)
                        pending.extend(
                            (ti + 5, c) for c in outproj_chunks(
                                qb, [ats[(qb, 0)], ats[(qb, 1)]], tags=tags))

            # drain: last PV flush, then everything left (the final q-block's
            # chunks alternate pf/po tags since the po banks are free now)
            pet, pkt0, pglen, ph, ppo = prev
            for j in range(pglen):
                kt = pkt0 + j
                nc.tensor.matmul(ppo[:], v_sb[:, kt, ph * HD:(ph + 1) * HD],
                                 pet[:, j, :], start=(kt == 0),
                                 stop=(kt == kt_n - 1))
            while pending:
                pending.pop(0)[1]()


def build_program(s_len=S):
    nc = bacc.Bacc("TRN2", target_bir_lowering=False, debug=False,
                   enable_asserts=False)
    st_n = s_len // 128
    io = {
        "ht": nc.dram_tensor("ht", [st_n, 128, CH, 128], BF16,
                             kind="ExternalInput").ap(),
        "wq": nc.dram_tensor("wq", [128, CH, DLOC], BF16,
                             kind="ExternalInput").ap(),
        "wk": nc.dram_tensor("wk", [128, CH, DLOC], BF16,
                             kind="ExternalInput").ap(),
        "wv": nc.dram_tensor("wv", [128, CH, DLOC], BF16,
                             kind="ExternalInput").ap(),
        "wo": nc.dram_tensor("wo", [128, HLOC, D], F32R,
                             kind="ExternalInput").ap(),
        "cgq": nc.dram_tensor("cgq", [s_len, DLOC], BF16,
                              kind="ExternalInput").ap(),
        "sgq": nc.dram_tensor("sgq", [s_len, DLOC], BF16,
                              kind="ExternalInput").ap(),
        "cgk": nc.dram_tensor("cgk", [s_len, DLOC], BF16,
                              kind="ExternalInput").ap(),
        "sgk": nc.dram_tensor("sgk", [s_len, DLOC], BF16,
                              kind="ExternalInput").ap(),
        "out": nc.dram_tensor("out", [s_len, D], F32R,
                              kind="ExternalOutput").ap(),
    }
    with tile.TileContext(nc) as tc:
        build(nc, tc, io, s_len)
    nc.compile()
    return nc


def prep_inputs(inputs, s_len=S):
    """Host-side preprocessing: transposed/tiled bf16 layouts + rope
    coefficient tables (g gains and the 1/sqrt(Hd) scale folded in),
    single head width, bf16."""
    bf16 = ml_dtypes.bfloat16
    hs = np.asarray(inputs["hidden_states"], np.float32).reshape(s_len, D)
    st_n = s_len // 128
    ht = np.ascontiguousarray(
        hs.reshape(st_n, 128, CH, 128).transpose(0, 3, 2, 1)).astype(bf16)

    fc = np.asarray(inputs["freqs_cis"], np.float32).reshape(s_len, HD)
    cos = np.cos(fc)
    sin = np.sin(fc)
    gq = np.asarray(inputs["gq"], np.float32)
    gk = np.asarray(inputs["gk"], np.float32)

    def coef(g, scale):
        cg = cos * g[None, :] * scale
        sg = np.empty_like(sin)
        sg[:, :64] = -sin[:, :64] * g[None, 64:] * scale
        sg[:, 64:] = sin[:, 64:] * g[None, :64] * scale
        cg2 = np.ascontiguousarray(np.tile(cg, (1, HLOC))).astype(bf16)
        sg2 = np.ascontiguousarray(np.tile(sg, (1, HLOC))).astype(bf16)
        return cg2, sg2

    cgq, sgq = coef(gq, SCL)
    cgk, sgk = coef(gk, 1.0)

    Wq = np.asarray(inputs["Wq"], np.float32)
    Wk = np.asarray(inputs["Wk"], np.float32)
    Wv = np.asarray(inputs["Wv"], np.float32)
    Wo = np.asarray(inputs["Wo"], np.float32)

    in_maps = []
    for c in range(NCORES):
        cols = slice(DLOC * c, DLOC * (c + 1))
        wq_c = np.ascontiguousarray(
            Wq[cols, :].T.reshape(CH, 128, DLOC).transpose(1, 0, 2)).astype(bf16)
        wk_c = np.ascontiguousarray(
            Wk[cols, :].T.reshape(CH, 128, DLOC).transpose(1, 0, 2)).astype(bf16)
        wv_c = np.ascontiguousarray(
            Wv[cols, :].T.reshape(CH, 128, DLOC).transpose(1, 0, 2)).astype(bf16)
        wo_c = np.ascontiguousarray(
            Wo[:, cols].T.reshape(HLOC, 128, D).transpose(1, 0, 2))
        in_maps.append({
            "ht": ht, "wq": wq_c, "wk": wk_c, "wv": wv_c, "wo": wo_c,
            "cgq": cgq, "sgq": sgq, "cgk": cgk, "sgk": sgk,
        })
    return in_maps


_CACHE = {}


def run_full(inputs, trace=False, **kw):
    if "nc" not in _CACHE:
        _CACHE["nc"] = build_program(S)
    nc = _CACHE["nc"]
    in_maps = prep_inputs(inputs, S)
    res = bass_utils.run_bass_kernel_spmd(
        nc, in_maps, core_ids=list(range(NCORES)), trace=trace, **kw)
    total = res.results[0]["out"].astype(np.float64)
    for c in range(1, NCORES):
        total += res.results[c]["out"]
    total += np.asarray(inputs["bo"], np.float64)[None, :]
    out = total.astype(np.float32).reshape(1, S, D)
    return out, res


def kernel(**inputs):
    out, _ = run_full(inputs, trace=False)
    return out
